# revision 5
# baseline (speedup 1.0000x reference)
"""Triplet-margin loss (hardest pos/neg over 512x16 clusters) on 8 trn2 cores.

Row-shard the [N, N] distance matrix (N=8192) across 8 cores. Each core holds
all embeddings X^T [128, 8192], key-rotated by rank*1024 so the same-cluster
diagonal block sits at a rank-independent position (one SPMD program for all
cores), and computes for its 1024 queries (128-query blocks):

    u[q, k] = 2 x_q.x_k - |x_k|^2 - 4096*same_cluster(q, k)

built entirely in PSUM by three fp32r matmuls per tile:
  main:  lhsT = 2*Xq^T [128, 128],  rhs = X^T chunk            (K=128)
  bias:  lhsT = -ones [2, 128],     rhs = [sq_hi; sq_lo] chunk (K=2,
         sq split hi/lo so fp32r rounding keeps ~1e-4 abs accuracy)
  mask:  lhsT = -4096*indicator [8, 128], rhs = indicator [8, 128]
         (same-cluster band = sum of 8 rank-1 outer products)

Then plain DVE reduces:
  max over the full chunk  -> -min_neg d^2 + |x_q|^2   (band sunk by -4096)
  min over the diag slice  -> -(max_pos d^2 - |x_q|^2) - 4096

loss = relu(sqrt(relu(sqq - negmax)) - sqrt(relu(sqq - slicemin - 4096)) + 1)
       ... with pos/neg roles: dan2 = sqq - negmax, dap2 = sqq - slicemin - 4096
"""

import json

import numpy as np

N = 8192          # total embeddings
D = 128           # embed dim
NCORES = 8
R = N // NCORES   # query rows per core (1024)
QB = 128          # query block width
NQB = R // QB     # query blocks per core (8)
CS = 2048         # key super-chunk width (4 PSUM banks)
NCS = N // CS     # super-chunks (4)
MM = 512          # single-matmul free width (1 PSUM bank)
BIG = 4096.0      # band sink; small enough that u keeps ~5e-4 abs precision
MARGIN = 1.0

_cached_nc = None


def _patch_wait_split(bass):
    """This container's walrus build rejects >1 sem-wait per instruction
    ("Too many sync wait commands"); split extra waits onto preceding
    same-engine NoOps at BIR-serialization time."""
    if getattr(bass.Bass, "_waitsplit_patched", False):
        return
    orig = bass.Bass.to_json_bytes

    def patched(self):
        bir = json.loads(orig(self))
        nop_id = [0]

        def fix_block(bb):
            new_insts = []
            for inst in bb.get("instructions", []):
                si = inst.get("sync_info") or {}
                waits = si.get("on_wait", [])
                while len(waits) > 1:
                    head, waits = waits[:1], waits[1:]
                    nop_id[0] += 1
                    new_insts.append({
                        "debug": inst.get("debug", 0),
                        "engine": inst["engine"],
                        "ins": [], "outs": [],
                        "name": f"I-waitsplit-{nop_id[0]}",
                        "opcode": "NoOp",
                        "sync_info": {"on_update": [], "on_wait": head},
                    })
                if si:
                    si["on_wait"] = waits
                new_insts.append(inst)
            bb["instructions"] = new_insts

        for m in bir.get("modules", [bir]):
            for fn in m.get("functions", []):
                for bb in fn.get("sb_blocks", []) + fn.get("blocks", []):
                    fix_block(bb)
        return json.dumps(bir).encode()

    bass.Bass.to_json_bytes = patched
    bass.Bass._waitsplit_patched = True


def _build_nc():
    global _cached_nc
    if _cached_nc is not None:
        return _cached_nc
    import concourse.bass as bass
    import concourse.mybir as mybir
    from concourse import tile

    _patch_wait_split(bass)
    f32 = mybir.dt.float32
    f32r = mybir.dt.float32r
    Alu = mybir.AluOpType
    Ax = mybir.AxisListType

    nc = bass.Bass()
    xt_d = nc.declare_dram_parameter("xt", [D, N], f32, isOutput=False)
    skhl_d = nc.declare_dram_parameter("skhl", [2, N], f32, isOutput=False)
    p2t_d = nc.declare_dram_parameter("p2t", [D, R], f32, isOutput=False)
    onesn_d = nc.declare_dram_parameter("onesn", [2, QB], f32, isOutput=False)
    ebig_d = nc.declare_dram_parameter("ebig", [8, QB], f32, isOutput=False)
    e01_d = nc.declare_dram_parameter("e01", [8, QB], f32, isOutput=False)
    sqq_d = nc.declare_dram_parameter("sqq", [128, NQB], f32, isOutput=False)
    out_d = nc.declare_dram_parameter("out", [NQB, QB], f32, isOutput=True)

    with tile.TileContext(nc) as tc:
        with tc.tile_pool(name="const", bufs=1) as const, \
             tc.tile_pool(name="stage", bufs=2) as stagep, \
             tc.tile_pool(name="psum", bufs=2, space="PSUM") as psum, \
             tc.tile_pool(name="acc", bufs=1) as accp:
            Act = mybir.ActivationFunctionType

            # fp32r matmul operands must be produced by a rounding engine op:
            # DMA fp32 into a staging tile, ACT-copy into the f32r tile.
            def load_rounded(name, parts, width, src_ap):
                dst = const.tile([parts, width], f32r, tag=name)
                st = stagep.tile([parts, width], f32, tag="stage")
                nc.gpsimd.dma_start(st[:], src_ap)
                nc.scalar.activation(dst[:], st[:], Act.Copy)
                return dst

            p2t = load_rounded("p2t", D, R, p2t_d[:])
            onesn = load_rounded("onesn", 2, QB, onesn_d[:])
            ebig = load_rounded("ebig", 8, QB, ebig_d[:])
            e01 = load_rounded("e01", 8, QB, e01_d[:])
            sqq = accp.tile([128, NQB], f32, tag="sqq")
            nc.gpsimd.dma_start(sqq[:], sqq_d[:])
            xts, skhls = [], []
            for c in range(NCS):
                xts.append(load_rounded(f"xt{c}", D, CS,
                                        xt_d[:, c * CS:(c + 1) * CS]))
                skhls.append(load_rounded(f"skhl{c}", 2, CS,
                                          skhl_d[:, c * CS:(c + 1) * CS]))

            chunkmaxs = accp.tile([128, NQB, NCS], f32, tag="chunkmaxs")
            slicemin = accp.tile([128, NQB], f32, tag="slicemin")

            for cs in range(NCS):
                for b in range(NQB):
                    pt = psum.tile([128, CS], f32, tag="pt")
                    lhsT = p2t[:, b * QB:(b + 1) * QB]
                    o = b * QB  # diag slice offset within chunk 0
                    mask_bank = o // MM
                    for j in range(CS // MM):
                        has_mask = (cs == 0 and j == mask_bank)
                        sl = slice(j * MM, (j + 1) * MM)
                        nc.tensor.matmul(pt[:, sl], lhsT, xts[cs][:, sl],
                                         start=True, stop=False)
                        nc.tensor.matmul(pt[:, sl], onesn, skhls[cs][:, sl],
                                         start=False, stop=not has_mask)
                        if has_mask:
                            nc.tensor.matmul(pt[:, o:o + QB], ebig, e01[:],
                                             start=False, stop=True,
                                             skip_group_check=True)
                    nc.vector.tensor_reduce(
                        chunkmaxs[:, b, cs:cs + 1], pt[:],
                        axis=Ax.X, op=Alu.max)
                    if cs == 0:
                        nc.vector.tensor_reduce(
                            slicemin[:, b:b + 1], pt[:, o:o + QB],
                            axis=Ax.X, op=Alu.min)

            negmax = accp.tile([128, NQB], f32, tag="negmax")
            nc.vector.tensor_reduce(negmax[:], chunkmaxs[:], axis=Ax.X,
                                    op=Alu.max)

            # dan^2 = sqq - negmax ; dap^2 = sqq - slicemin - BIG
            dan2 = accp.tile([128, NQB], f32, tag="dan2")
            nc.vector.tensor_sub(dan2[:], sqq[:], negmax[:])
            dan2c = accp.tile([128, NQB], f32, tag="dan2c")
            nc.vector.tensor_scalar_max(dan2c[:], dan2[:], 0.0)
            dan = accp.tile([128, NQB], f32, tag="dan")
            nc.scalar.sqrt(dan[:], dan2c[:])

            dap2 = accp.tile([128, NQB], f32, tag="dap2")
            nc.vector.tensor_sub(dap2[:], sqq[:], slicemin[:])
            dap2b = accp.tile([128, NQB], f32, tag="dap2b")
            nc.vector.tensor_scalar_add(dap2b[:], dap2[:], -BIG)
            dap2c = accp.tile([128, NQB], f32, tag="dap2c")
            nc.vector.tensor_scalar_max(dap2c[:], dap2b[:], 0.0)
            dap = accp.tile([128, NQB], f32, tag="dap")
            nc.scalar.sqrt(dap[:], dap2c[:])

            diff = accp.tile([128, NQB], f32, tag="diff")
            nc.vector.tensor_sub(diff[:], dap[:], dan[:])
            loss = accp.tile([128, NQB], f32, tag="loss")
            nc.scalar.activation(loss[:], diff[:], Act.Relu,
                                 bias=MARGIN, scale=1.0)
            for b in range(NQB):
                nc.gpsimd.dma_start(out_d[b, :], loss[:, b:b + 1])

    _cached_nc = nc
    return nc


def _bf16_trunc(x):
    b = np.ascontiguousarray(x, dtype=np.float32).view(np.uint32)
    return (b & np.uint32(0xFFFF0000)).view(np.float32)


def _make_in_maps(batch):
    X = np.ascontiguousarray(batch.reshape(N, D), dtype=np.float32)
    XT = np.ascontiguousarray(X.T)
    sq = np.einsum("nd,nd->n", X, X).astype(np.float32)

    gidx = np.arange(QB) // 16
    onesn = np.full((2, QB), -1.0, np.float32)
    ebig = np.where(gidx[None, :] == np.arange(8)[:, None], -BIG, 0.0
                    ).astype(np.float32)
    e01 = (gidx[None, :] == np.arange(8)[:, None]).astype(np.float32)

    in_maps = []
    for r in range(NCORES):
        roll = (np.arange(N) + r * R) % N
        xt_r = np.ascontiguousarray(XT[:, roll])
        sq_r = sq[roll]
        hi = _bf16_trunc(sq_r)
        skhl = np.ascontiguousarray(np.stack([hi, sq_r - hi]))
        p2t = np.ascontiguousarray((2.0 * X[r * R:(r + 1) * R]).T
                                   ).astype(np.float32)
        sqq = np.ascontiguousarray(
            sq[r * R:(r + 1) * R].reshape(NQB, QB).T).astype(np.float32)
        in_maps.append({"xt": xt_r, "skhl": skhl, "p2t": p2t,
                        "onesn": onesn, "ebig": ebig, "e01": e01,
                        "sqq": sqq})
    return in_maps


def kernel(batch):
    from concourse.bass_utils import run_bass_kernel_spmd

    batch = np.asarray(batch, dtype=np.float32)
    nc = _build_nc()
    in_maps = _make_in_maps(batch)
    res = run_bass_kernel_spmd(nc, in_maps, list(range(NCORES)))
    out = np.concatenate(
        [np.asarray(res.results[r]["out"], dtype=np.float32).reshape(-1)
         for r in range(NCORES)])
    return out
